# revision 37
# baseline (speedup 1.0000x reference)
"""Trainium2 Bass kernel for nn_CABlock_26912265077025.

Architecture: CA-gating block (pools -> conv -> sigmoid gates -> x*gd*gh*gw)
followed by a 12000->4096->512->3 MLP and row L2-normalization.

Strategy: pure data parallelism over the batch across 8 NeuronCores
(512 rows each). The dominant 4096x12032 matmul runs as fp8(e4m3)
DoubleRow matmuls (2 k-tiles per instruction at 0.5 cycles/row) with
same-scale hi/lo error compensation on BOTH operands:
  W = w1*2^7  ~ A + Cq      (A = Q8(W), Cq = Q8(W - A))
  Z = z *2^5  ~ zh + zl     (zh = Q8(Z), zl = Q8(Z - zh))
  W@Z ~ A@zh + A@zl + Cq@zh   (3 DoubleRow streams, one PSUM acc)
Gate matmuls are also fp8 DoubleRow (selection is exact in fp8; Lp is
hi/lo-compensated with hi and lo packed into the two DR slots).
Everything else (pools/mm2) runs in fp16; mm3 + normalize in f32.
x is streamed twice (pools pass, gating pass) to keep SBUF under budget.
The first PASS0 mm1 m-tiles run k-outer DURING gating (staggered joins)
to fill the PE while ACT/DVE/Pool produce zh/zl. DMAs are spread across
engine queues: a DMA holds its issuing queue for the whole transfer here.
"""

from contextlib import ExitStack, nullcontext

import numpy as np
import ml_dtypes

import concourse.bass as bass
import concourse.mybir as mybir
import concourse.tile as tile
from concourse import bacc
from concourse.bass_utils import run_bass_kernel_spmd

N_CORES = 8
B_TOT = 4096
BS = B_TOT // N_CORES           # 512 batch rows per core
F = 12000                       # 3*10*10*40 flattened features
NK = 94                         # ceil(F/128) k-tiles
NKP = NK // 2                   # 47 DoubleRow k-pairs
FP = NK * 128                   # 12032 (rows F..FP-1 zero-padded)
H1, H2 = 4096, 512
NM1 = H1 // 128                 # 32 mm1 output tiles
NK2, NM2 = H1 // 128, H2 // 128 # 32, 4
SELG = 8                        # ssel k-tiles per DMA group
NSG = (NK + SELG - 1) // SELG   # 12
WPG = 8                         # wpool (and x) k-tiles per group, pools pass
NWPG = (NK + WPG - 1) // WPG    # 12
XCG = 4                         # x k-tiles per DMA in gating pass
PASS0 = 6                       # mm1 m-tiles run k-outer during gating
CH = 12                         # w1 chunk size in k-pairs
NCH = (NKP + CH - 1) // CH      # 4 chunks per m-tile
P0_DELAY = 2                    # pairs of pipeline margin before consuming
P0_RATE = 6                     # max extra catch-up pairs per production pair

SW = 2.0**7                     # w1 fp8 scale
SZ = 2.0**5                     # z fp8 scale
SLP = 16.0                      # Lp fp8 scale
UNSCALE = 1.0 / (SW * SZ)

f32 = mybir.dt.float32
f16 = mybir.dt.float16
f8 = mybir.dt.float8e4
AF = mybir.ActivationFunctionType
DR = mybir.MatmulPerfMode.DoubleRow
E4M3 = ml_dtypes.float8_e4m3

_NC_CACHE = {}


def build_nc():
    nc = bacc.Bacc(None, target_bir_lowering=False)

    xt_d = nc.dram_tensor("xt", [NK, 128, BS], f16, kind="ExternalInput")
    w1a_d = nc.dram_tensor("w1a", [NM1, 128, NKP, 2, 128], f8, kind="ExternalInput")
    w1c_d = nc.dram_tensor("w1c", [NM1, 128, NKP, 2, 128], f8, kind="ExternalInput")
    wpool_d = nc.dram_tensor("wpool", [NWPG, 128, WPG * 50], f16, kind="ExternalInput")
    rmat_d = nc.dram_tensor("rmat", [50, 180], f16, kind="ExternalInput")
    ssela_d = nc.dram_tensor("ssela", [NSG, 128, SELG, 2, 128], f8, kind="ExternalInput")
    sselb_d = nc.dram_tensor("sselb", [NSG, 52, SELG, 2, 128], f8, kind="ExternalInput")
    w2k_d = nc.dram_tensor("w2k", [NK2, 128, NM2, 128], f16, kind="ExternalInput")
    w3h_d = nc.dram_tensor("w3h", [128, NM2, 3], f16, kind="ExternalInput")
    b1_d = nc.dram_tensor("b1g", [128, NM1], f32, kind="ExternalInput")
    b2_d = nc.dram_tensor("b2g", [128, NM2], f32, kind="ExternalInput")
    b3_d = nc.dram_tensor("b3g", [3, 1], f32, kind="ExternalInput")
    out_d = nc.dram_tensor("out", [3, BS], f32, kind="ExternalOutput")

    with tile.TileContext(nc) as tc, ExitStack() as ctx:
        consts = ctx.enter_context(tc.tile_pool(name="consts", bufs=1))

        b1_sb = consts.tile([128, NM1], f32)
        b2_sb = consts.tile([128, NM2], f32)
        b3_sb = consts.tile([3, 1], f32)
        w3_sb = consts.tile([128, NM2, 3], f16)
        ones31 = consts.tile([3, 1], f16)
        ones13 = consts.tile([1, 3], f16)
        lnsz = consts.tile([128, 1], f32)
        nc.sync.dma_start(b1_sb[:], b1_d[:])
        nc.sync.dma_start(b2_sb[:], b2_d[:])
        nc.sync.dma_start(b3_sb[:], b3_d[:])
        nc.sync.dma_start(w3_sb[:], w3h_d[:])
        nc.any.memset(ones31[:], 1.0)
        nc.any.memset(ones13[:], 1.0)
        nc.any.memset(lnsz[:], float(np.log(SZ)))

        # z hi/lo pair tiles persist through phase D
        zstack = ExitStack()
        zhp = zstack.enter_context(tc.tile_pool(name="zhp", bufs=NKP))
        zlp = zstack.enter_context(tc.tile_pool(name="zlp", bufs=NKP))
        zh_pairs = []
        zl_pairs = []

        # w1 streamed as k-chunks of CH pairs (small tiles avoid the WAR
        # deadlock of full-tile buffers when pass0 m-tiles stay live all of C)
        dstack = ExitStack()
        w1ap = dstack.enter_context(tc.tile_pool(name="w1ap", bufs=10))
        w1cp = dstack.enter_context(tc.tile_pool(name="w1cp", bufs=10))
        z1p = dstack.enter_context(tc.tile_pool(name="z1p", bufs=8))
        w2p = dstack.enter_context(tc.tile_pool(name="w2p", bufs=8))
        wa_ch = {}
        wc_ch = {}

        def fetch_chunk(m, c, qa=None, qc=None, wait_ms=None):
            lo, hi = c * CH, min((c + 1) * CH, NKP)
            with tc.tile_wait_until(wait_ms) if wait_ms is not None else nullcontext():
                wa = w1ap.tile([128, CH, 2, 128], f8, tag="wa")
                (qa or nc.sync).dma_start(wa[:, : hi - lo], w1a_d[m, :, lo:hi])
                wc = w1cp.tile([128, CH, 2, 128], f8, tag="wc")
                (qc or nc.scalar).dma_start(wc[:, : hi - lo], w1c_d[m, :, lo:hi])
                wa_ch[(m, c)] = wa
                wc_ch[(m, c)] = wc

        def mm1_pair(m, kp, acc):
            c, j = divmod(kp, CH)
            wa, wc = wa_ch[(m, c)], wc_ch[(m, c)]
            nc.tensor.matmul(
                acc[:], wa[:, j], zh_pairs[kp][:],
                start=(kp == 0), stop=False, perf_mode=DR,
            )
            nc.tensor.matmul(
                acc[:], wa[:, j], zl_pairs[kp][:],
                start=False, stop=False, perf_mode=DR,
            )
            nc.tensor.matmul(
                acc[:], wc[:, j], zh_pairs[kp][:],
                start=False, stop=(kp == NKP - 1), perf_mode=DR,
            )
            if kp == NKP - 1 or (kp + 1) % CH == 0:
                wa_ch.pop((m, c))
                wc_ch.pop((m, c))

        p0_progress = [0] * PASS0
        p0_limit = [0] * PASS0
        p0_join = [0, 0, 1, 2, 3, 4][:PASS0]  # near-immediate joins: PE backlog keeps it saturated

        # chunk-fetch schedule: issue ~6 pairs before first consumption
        from collections import defaultdict
        fetch_at = defaultdict(list)
        for j in range(PASS0):
            for c in range(NCH):
                start_kp = max(p0_join[j], c * CH)
                if c == 0:
                    issue = -1 if j < 2 else j - 2  # spread the c0 flood
                else:
                    issue = start_kp - 6
                issue = min(issue, start_kp - 1) if issue >= 0 else issue
                fetch_at[issue].append((j, c))

        # ---------------- Phase A: pools — ypre = wpool^T @ x  (fp16)
        # Phase B: y = relu(ypre); T = rmat^T @ y; Lp = ln(1+exp(-T)) -> fp8 hi/lo
        Lpa8 = consts.tile([128, 2, BS], f8)
        Lpb8 = consts.tile([52, 2, BS], f8)
        with (
            tc.tile_pool(name="xa", bufs=3) as xa,
            tc.tile_pool(name="wpp", bufs=2) as wpp,
            tc.tile_pool(name="psy", bufs=1, space="PSUM") as psy,
            tc.tile_pool(name="yp", bufs=1) as yp,
            tc.tile_pool(name="pst", bufs=1, space="PSUM") as pst,
        ):
            ypre = psy.tile([50, BS], f32)
            for g in range(NWPG):
                cnt = min(WPG, NK - g * WPG)
                wpt = wpp.tile([128, WPG, 50], f16, tag="wp")
                nc.gpsimd.dma_start(
                    wpt[:, :cnt, :],
                    wpool_d[g, :, : cnt * 50].rearrange("p (k c) -> p k c", c=50),
                )
                xg = xa.tile([128, WPG, BS], f16, tag="xa")
                q = nc.sync if g % 2 == 0 else nc.scalar
                q.dma_start(
                    xg[:, :cnt, :],
                    xt_d[g * WPG : g * WPG + cnt].rearrange("k p b -> p k b"),
                )
                for i in range(cnt):
                    k = g * WPG + i
                    nc.tensor.matmul(
                        ypre[:], wpt[:, i, :], xg[:, i, :],
                        start=(k == 0), stop=(k == NK - 1),
                    )

            # w1 chunk prefetch only after phase-A x loads: DMA transfers
            # serialize globally, so big loads must not cut ahead
            for m, c in fetch_at.pop(-1, []):
                fetch_chunk(m, c, qa=nc.sync, qc=nc.scalar, wait_ms=0.038)

            y_sb = yp.tile([50, BS], f16)
            rm_sb = yp.tile([50, 180], f16)
            nc.scalar.dma_start(rm_sb[:], rmat_d[:])
            nc.scalar.activation(y_sb[:], ypre[:], AF.Relu)
            Ta = pst.tile([128, BS], f32, tag="T")
            Tb = pst.tile([52, BS], f32, tag="T2")
            nc.tensor.matmul(Ta[:], rm_sb[:, 0:128], y_sb[:])
            nc.tensor.matmul(Tb[:], rm_sb[:, 128:180], y_sb[:])
            nc.scalar.activation(Ta[:], Ta[:], AF.Exp, scale=-1.0)
            nc.scalar.activation(Tb[:], Tb[:], AF.Exp, scale=-1.0)
            Lpaf = yp.tile([128, BS], f16)
            Lpbf = yp.tile([52, BS], f16)
            nc.scalar.activation(Lpaf[:], Ta[:], AF.Ln, bias=1.0)
            nc.scalar.activation(Lpbf[:], Tb[:], AF.Ln, bias=1.0)
            # Lp8 = (hi, lo) at scale SLP, same-scale residual in slot 1
            nc.scalar.activation(Lpa8[:, 0, :], Lpaf[:], AF.Copy, scale=SLP)
            nc.scalar.activation(Lpb8[:, 0, :], Lpbf[:], AF.Copy, scale=SLP)
            nc.vector.scalar_tensor_tensor(
                Lpa8[:, 1, :], Lpaf[:], SLP, Lpa8[:, 0, :],
                mybir.AluOpType.mult, mybir.AluOpType.subtract,
            )
            nc.vector.scalar_tensor_tensor(
                Lpb8[:, 1, :], Lpbf[:], SLP, Lpb8[:, 0, :],
                mybir.AluOpType.mult, mybir.AluOpType.subtract,
            )

        # ---------------- Phase C: G = SZ*exp(-(ssel^T@Lp)/SLP); Z = x*G -> zh+zl
        #                  with PASS0 mm1 m-tiles consuming pairs k-outer
        psm0stack = ExitStack()
        if PASS0:
            psm0 = psm0stack.enter_context(
                tc.tile_pool(name="psm0", bufs=1, space="PSUM")
            )
            acc0s = [psm0.tile([128, BS], f32, tag=f"p0_{i}", name=f"acc0_{i}") for i in range(PASS0)]
        else:
            acc0s = []
        def emit_pass0(m, upto):
            acc = acc0s[m]
            while p0_progress[m] < upto:
                mm1_pair(m, p0_progress[m], acc)
                p0_progress[m] += 1

        with (
            tc.tile_pool(name="xc", bufs=2) as xc,
            tc.tile_pool(name="sselp", bufs=2) as sselp,
            tc.tile_pool(name="psg", bufs=2, space="PSUM") as psg,
            tc.tile_pool(name="gtp", bufs=2) as gtp,
            tc.tile_pool(name="ztp", bufs=2) as ztp,
        ):
            sqa = sqb = None
            sq_g = -1
            xg = None
            for kp in range(NKP):
                for i in range(2):
                    k = 2 * kp + i
                    if k % SELG == 0:
                        sq_g = k // SELG
                        sqa = sselp.tile([128, SELG, 2, 128], f8, tag="sqa")
                        nc.sync.dma_start(sqa[:], ssela_d[sq_g])
                        sqb = sselp.tile([52, SELG, 2, 128], f8, tag="sqb")
                        nc.sync.dma_start(sqb[:], sselb_d[sq_g])
                    if k % XCG == 0:
                        xn = min(XCG, NK - k)
                        xg = xc.tile([128, XCG, BS], f16, tag="xc")
                        nc.sync.dma_start(
                            xg[:, :xn, :],
                            xt_d[k : k + xn].rearrange("k p b -> p k b"),
                        )
                    if i == 0:
                        zh_t = zhp.tile([128, 2, BS], f8, tag="zh")
                        zl_t = zlp.tile([128, 2, BS], f8, tag="zl")
                        zh_pairs.append(zh_t)
                        zl_pairs.append(zl_t)
                    j = k % SELG
                    gp = psg.tile([128, BS], f32, tag="g")
                    nc.tensor.matmul(
                        gp[:], sqa[:, j], Lpa8[:],
                        start=True, stop=False, perf_mode=DR,
                    )
                    nc.tensor.matmul(
                        gp[:], sqb[:, j], Lpb8[:],
                        start=False, stop=True, perf_mode=DR,
                    )
                    gt = gtp.tile([128, BS], f16, tag="G")
                    nc.scalar.activation(
                        gt[:], gp[:], AF.Exp, scale=-1.0 / SLP, bias=lnsz[:, 0:1]
                    )
                    zt = ztp.tile([128, BS], f16, tag="Zt")
                    nc.vector.tensor_mul(zt[:], xg[:, k % XCG, :], gt[:])
                    # zh/zl engine rotation balances ACT (exp-loaded), DVE
                    # (mul-loaded) and Pool (weak but otherwise idle)
                    r = k % 8
                    if r in (0, 1, 4, 5, 7):         # zh: Pool x5
                        nc.gpsimd.tensor_copy(zh_t[:, i, :], zt[:])
                    elif r in (2, 6):                # zh: ACT x2
                        nc.scalar.activation(zh_t[:, i, :], zt[:], AF.Copy)
                    else:                            # zh: DVE x1
                        nc.vector.tensor_copy(zh_t[:, i, :], zt[:])
                    zl_eng = nc.gpsimd if r in (2, 6) else nc.vector
                    zl_eng.tensor_tensor(
                        zl_t[:, i, :], zt[:], zh_t[:, i, :],
                        mybir.AluOpType.subtract,
                    )
                for m, c in fetch_at.pop(kp, []):
                    fetch_chunk(m, c)
                # staggered pass0 joins, P0_DELAY pairs behind production,
                # catch-up spread at P0_RATE extra pairs per production pair
                for m in range(PASS0):
                    if kp >= p0_join[m]:
                        p0_limit[m] += 1 + P0_RATE
                        emit_pass0(m, min(max(kp + 1 - P0_DELAY, 0), p0_limit[m]))

            for m in range(PASS0):
                emit_pass0(m, NKP)

        # ---------------- Phase D: z1 = relu(UNSCALE*acc + b1); mm2 (fp16)
        pending = []

        def retire_m(m, acc):
            z1t = z1p.tile([128, BS], f16, tag="z1")
            nc.scalar.activation(
                z1t[:], acc[:], AF.Relu, bias=b1_sb[:, m : m + 1], scale=UNSCALE
            )
            w2t = w2p.tile([128, NM2, 128], f16, tag="w2")
            nc.scalar.dma_start(w2t[:], w2k_d[m])
            pending.append((m, z1t, w2t))

        for m in range(PASS0):
            retire_m(m, acc0s[m])
        psm0stack.close()

        psm2 = dstack.enter_context(tc.tile_pool(name="psm2", bufs=1, space="PSUM"))
        acc2s = [
            psm2.tile([128, BS], f32, tag=f"mm2_{m2}", name=f"acc2_{m2}")
            for m2 in range(NM2)
        ]
        psmstack = ExitStack()
        psm = psmstack.enter_context(tc.tile_pool(name="psm", bufs=2, space="PSUM"))

        def emit_mm2(pending):
            for k2, z1t, w2t in pending:
                for m2 in range(NM2):
                    nc.tensor.matmul(
                        acc2s[m2][:],
                        w2t[:, m2, :],
                        z1t[:],
                        start=(k2 == 0),
                        stop=(k2 == NK2 - 1),
                        skip_group_check=True,
                    )
            pending.clear()

        for mm in (PASS0, PASS0 + 1):
            if mm < NM1:
                for c in range(NCH):
                    fetch_chunk(mm, c)
        for m in range(PASS0, NM1):
            acc = psm.tile([128, BS], f32, tag="mm1")
            for kp in range(NKP):
                mm1_pair(m, kp, acc)
                if kp == 2:
                    # deferred mm2: the pending z1 relus have drained off ACT
                    emit_mm2(pending)
                if kp == 24 and m + 2 < NM1:
                    for c in range(NCH):
                        fetch_chunk(m + 2, c)
            retire_m(m, acc)
        emit_mm2(pending)
        psmstack.close()

        # ---------------- Phase E: z2 = relu(acc2 + b2); F: mm3 + normalize
        z2_tiles = []
        with (
            tc.tile_pool(name="z2p", bufs=NM2) as z2p,
            tc.tile_pool(name="tailp", bufs=1) as tailp,
            tc.tile_pool(name="psf", bufs=1, space="PSUM") as psf,
        ):
            for m2 in range(NM2):
                z2t = z2p.tile([128, BS], f16, tag="z2")
                nc.scalar.activation(
                    z2t[:], acc2s[m2][:], AF.Relu, bias=b2_sb[:, m2 : m2 + 1]
                )
                z2_tiles.append(z2t)

            acc3 = psf.tile([3, BS], f32, tag="f")
            for k3 in range(NM2):
                nc.tensor.matmul(
                    acc3[:], w3_sb[:, k3, :], z2_tiles[k3][:],
                    start=(k3 == 0), stop=(k3 == NM2 - 1),
                )
            z3 = tailp.tile([3, BS], f32)
            nc.vector.tensor_scalar_add(z3[:], acc3[:], b3_sb[:])
            sq = tailp.tile([3, BS], f16)
            nc.scalar.activation(sq[:], z3[:], AF.Square)
            sps = psf.tile([1, BS], f32, tag="f2")
            nc.tensor.matmul(sps[:], ones31[:], sq[:])
            # 1/max(sqrt(s), 1e-12) = min(exp(-0.5*ln(s)), 1e12)
            lns = tailp.tile([1, BS], f32)
            nc.scalar.activation(lns[:], sps[:], AF.Ln)
            inv = tailp.tile([1, BS], f16)
            nc.scalar.activation(inv[:], lns[:], AF.Exp, scale=-0.5)
            nc.vector.tensor_scalar_min(inv[:], inv[:], 1e12)
            inv3 = psf.tile([3, BS], f32, tag="f3")
            nc.tensor.matmul(inv3[:], ones13[:], inv[:])
            outt = tailp.tile([3, BS], f32)
            nc.vector.tensor_mul(outt[:], z3[:], inv3[:])
            nc.sync.dma_start(out_d[:], outt[:])

        dstack.close()
        zstack.close()

    nc.compile()
    return nc


def _prep_shared(conv_w, F_w, w1, b1, w2, b2, w3, b3):
    """Host-side weight layouts shared by all cores."""
    fa = np.arange(F)
    c_idx = fa // 4000
    d_idx = (fa // 400) % 10
    h_idx = (fa // 40) % 10
    w_idx = fa % 40

    # pooled conv: y_pre[j] = sum_f wp[f, j] * x^T[f, b]
    wp = np.zeros((NWPG * WPG * 128, 50), np.float32)
    wp[fa, h_idx] = conv_w[c_idx] / 400.0
    wp[fa, 10 + w_idx] = conv_w[c_idx] / 100.0
    wpool = np.ascontiguousarray(
        wp.reshape(NWPG, WPG, 128, 50).transpose(0, 2, 1, 3)
    ).astype(np.float16)

    # rmat: T[r, b] = sum_j rmat[j, r] * y[j, b]
    rm = np.zeros((50, 180), np.float32)
    cc10 = np.repeat(np.arange(3), 10)
    rm[np.tile(np.arange(10), 3), np.arange(30)] = F_w[cc10]
    rm[np.tile(np.arange(10), 3), 30 + np.arange(30)] = F_w[cc10]
    cc40 = np.repeat(np.arange(3), 40)
    rm[10 + np.tile(np.arange(40), 3), 60 + np.arange(120)] = F_w[cc40]
    rm = rm.astype(np.float16)

    # selection: logG[f] = -(sel[f, :] @ (Lp_hi + Lp_lo))/SLP; both DR slots
    # carry the same selection matrix (slot 0 hits Lp_hi, slot 1 Lp_lo)
    sel = np.zeros((FP, 180), np.float32)
    sel[fa, c_idx * 10 + d_idx] = 1.0
    sel[fa, 30 + c_idx * 10 + h_idx] = 1.0
    sel[fa, 60 + c_idx * 40 + w_idx] = 1.0
    selp = np.zeros((NSG * SELG * 128, 180), np.float32)
    selp[:FP] = sel
    selp = selp.reshape(NSG, SELG, 128, 180).transpose(0, 3, 1, 2)  # [g, r, j, p]
    ssela = np.zeros((NSG, 128, SELG, 2, 128), np.float32)
    ssela[:, :, :, 0, :] = selp[:, 0:128]
    ssela[:, :, :, 1, :] = selp[:, 0:128]
    sselb = np.zeros((NSG, 52, SELG, 2, 128), np.float32)
    sselb[:, :, :, 0, :] = selp[:, 128:180]
    sselb[:, :, :, 1, :] = selp[:, 128:180]
    ssela = np.ascontiguousarray(ssela).astype(E4M3)
    sselb = np.ascontiguousarray(sselb).astype(E4M3)

    # w1 hi/lo fp8 split: W = w1*SW ~ A + Cq, laid out per m-tile as
    # [m][p, kp, slot, mm] = val[m*128+mm, (2*kp+slot)*128+p]
    w1p_ = np.zeros((H1, FP), np.float32)
    w1p_[:, :F] = w1 * SW
    A = np.clip(w1p_, -240, 240).astype(E4M3)
    Cq = (w1p_ - A.astype(np.float32)).astype(E4M3)

    def w1_layout(t):
        r = t.reshape(NM1, 128, NKP, 2, 128)
        return np.ascontiguousarray(r.transpose(0, 4, 2, 3, 1))

    w1a = w1_layout(A)
    w1c = w1_layout(Cq)

    w2k = np.ascontiguousarray(
        w2.reshape(NM2, 128, NK2, 128).transpose(2, 3, 0, 1)
    ).astype(np.float16)
    w3h = np.ascontiguousarray(w3.reshape(3, NM2, 128).transpose(2, 1, 0)).astype(np.float16)

    return {
        "wpool": wpool,
        "rmat": rm,
        "ssela": ssela,
        "sselb": sselb,
        "w1a": w1a,
        "w1c": w1c,
        "w2k": w2k,
        "w3h": w3h,
        "b1g": np.ascontiguousarray(b1.reshape(NM1, 128).T),
        "b2g": np.ascontiguousarray(b2.reshape(NM2, 128).T),
        "b3g": np.ascontiguousarray(b3.reshape(3, 1)),
    }


def make_in_maps(x, conv_w, F_w, w1, b1, w2, b2, w3, b3):
    x = np.asarray(x, np.float32).reshape(B_TOT, F)
    shared = _prep_shared(
        np.asarray(conv_w, np.float32).reshape(3),
        np.asarray(F_w, np.float32).reshape(3),
        np.asarray(w1, np.float32),
        np.asarray(b1, np.float32),
        np.asarray(w2, np.float32),
        np.asarray(b2, np.float32),
        np.asarray(w3, np.float32),
        np.asarray(b3, np.float32),
    )
    in_maps = []
    for c in range(N_CORES):
        xs = x[c * BS : (c + 1) * BS]
        xt = np.zeros((FP, BS), np.float16)
        xt[:F] = xs.T.astype(np.float16)
        m = dict(shared)
        m["xt"] = xt.reshape(NK, 128, BS)
        in_maps.append(m)
    return in_maps


def get_nc():
    if "nc" not in _NC_CACHE:
        _NC_CACHE["nc"] = build_nc()
    return _NC_CACHE["nc"]


def kernel(**inputs) -> np.ndarray:
    nc = get_nc()
    in_maps = make_in_maps(**inputs)
    res = run_bass_kernel_spmd(nc, in_maps, core_ids=list(range(N_CORES)))
    out = np.concatenate([r["out"] for r in res.results], axis=1)  # [3, 4096]
    return np.ascontiguousarray(out.T, dtype=np.float32)
